# revision 35
# baseline (speedup 1.0000x reference)
"""Trainium2 Bass kernel (final): batched inverse of homogeneous affines.

DVE-saturated design (HW-traced: DVE ~95% duty, every op at its cost-model
speed, fp16 ops confirmed in 2x perf mode).
  - cross products / z sub / col3 dot / e1 stay f32: fp16 input rounding
    breaks the pointwise rel-err gate on cancellation-heavy cofactors
    (measured rel.max 4.4 with fp16 z on the fixed seed)
  - fp16 2x mode where the error stays relative per element: det dot,
    inv = z*rdet, c3 = e1*rdet (total rel.max 6.3e-3, 3x under the gate)
  - ACT does wrap-row copy, tneg, f32<->fp16 conversions, rdet
    replication, and issues output DMA on its own HWDGE ring
  - GpSimd unused: it shares the DVE SBUF port, so elementwise offload
    there is fully serialized with DVE (measured) - strictly negative
  - stride-0 broadcast APs fuse the w and inv scale muls (2x retained)
DVE FIFO order is a dependency ladder (cross, sub, w, tm, d1, det, recip,
e1, inv16, c3) so ACT work never stalls it; ~20% run-to-run clock
throttling exists on the part - compare runs via per-op durations.
"""

import numpy as np

B = 4_194_304
NCORES = 8
BL = B // NCORES
P = 128
CHUNKS = [96] + [496] * 7 + [368, 160]  # sums to BL/P = 4096
assert sum(CHUNKS) == BL // P


def _V(base_ap, off, dims):
    import concourse.bass as bass

    return bass.AP(
        base_ap.tensor,
        base_ap.offset + off,
        [list(base_ap.ap[0])] + [[int(s), int(n)] for s, n in dims],
    )


def build_nc(bl=BL, chunks=None):
    import concourse.bass as bass
    import concourse.bacc as bacc
    import concourse.mybir as mybir
    from concourse.tile import TileContext

    chunks = chunks or CHUNKS
    assert sum(chunks) * P == bl
    f32 = mybir.dt.float32
    f16 = mybir.dt.float16

    nc = bacc.Bacc()
    trf = nc.declare_dram_parameter("trf", [12, bl], f32, isOutput=False)
    out = nc.declare_dram_parameter("out", [12, bl], f32, isOutput=True)
    V = nc.vector
    S = nc.scalar

    bases = []
    acc = 0
    for c in chunks:
        bases.append(acc)
        acc += P * c

    def dram_in_ap(base, c, row0, nrows):
        return bass.AP(trf.ap().tensor, base + row0 * bl,
                       [[c, P], [bl, nrows], [1, c]])

    def dram_out_ap(base, c, row0, nrows):
        return bass.AP(out.ap().tensor, base + row0 * bl,
                       [[c, P], [bl, nrows], [1, c]])

    with TileContext(nc) as tc:
        with (
            tc.tile_pool(name="io", bufs=2) as io,
            tc.tile_pool(name="zz", bufs=2) as zz,
            tc.tile_pool(name="s1", bufs=1) as s1,
            tc.tile_pool(name="iv", bufs=1) as iv,
        ):

            def prep(n):
                c = chunks[n]
                tin = io.tile([P, 15 * c], f32, tag="tin")
                nc.sync.dma_start(out=_V(tin, 0, [(1, 9 * c)]),
                                  in_=dram_in_ap(bases[n], c, 0, 9))
                # rows 9..11 = DRAM rows 0..2 again (colpos_3 wrap) so the
                # cross g=2 operand needs no ACT round trip
                nc.sync.dma_start(out=_V(tin, 9 * c, [(1, 3 * c)]),
                                  in_=dram_in_ap(bases[n], c, 0, 3))
                nc.sync.dma_start(out=_V(tin, 12 * c, [(1, 3 * c)]),
                                  in_=dram_in_ap(bases[n], c, 9, 3))
                return {"tin": tin, "c": c, "n": n}

            def wrap(st):
                # ACT: a16 = fp16(col1) for the 2x det dot
                c, tin = st["c"], st["tin"]
                a16 = s1.tile([P, 3 * c], f16, tag="a16")
                S.copy(_V(a16, 0, [(1, 3 * c)]), _V(tin, 0, [(1, 3 * c)]))
                st["a16"] = a16

            def cross(st):
                # DVE f32: z[3g+j] = (colpos_g x colpos_{g+1})[j]
                # positive products land in the z tile; sub is in-place
                c, tin = st["c"], st["tin"]
                ng = s1.tile([P, 9 * c], f32, tag="ngw")
                z = zz.tile([P, 9 * c], f32, tag="z")
                for j in range(3):
                    V.tensor_mul(
                        _V(z, j * c, [(3 * c, 3), (1, c)]),
                        _V(tin, ((j + 1) % 3) * c, [(3 * c, 3), (1, c)]),
                        _V(tin, (3 + (j + 2) % 3) * c, [(3 * c, 3), (1, c)]),
                    )
                for j in range(3):
                    V.tensor_mul(
                        _V(ng, j * c, [(3 * c, 3), (1, c)]),
                        _V(tin, ((j + 2) % 3) * c, [(3 * c, 3), (1, c)]),
                        _V(tin, (3 + (j + 1) % 3) * c, [(3 * c, 3), (1, c)]),
                    )
                V.tensor_sub(_V(z, 0, [(1, 9 * c)]),
                             _V(z, 0, [(1, 9 * c)]),
                             _V(ng, 0, [(1, 9 * c)]))
                st["z"] = z

            def tneg(st):
                # ACT: rows 9..11 (wrap copy, dead after cross) := -t  (f32)
                c, tin = st["c"], st["tin"]
                S.mul(_V(tin, 9 * c, [(1, 3 * c)]),
                      _V(tin, 12 * c, [(1, 3 * c)]), -1.0)

            def conv_z(st):
                c, z = st["c"], st["z"]
                z16 = s1.tile([P, 9 * c], f16, tag="z16")
                S.copy(_V(z16, 0, [(1, 9 * c)]), _V(z, 0, [(1, 9 * c)]))
                st["z16"] = z16

            def col3_w(st):
                # DVE f32: w = z * (-t)
                c, tin, z = st["c"], st["tin"], st["z"]
                w = s1.tile([P, 9 * c], f32, tag="ngw")
                V.tensor_mul(
                    _V(w, 0, [(3 * c, 3), (1, 3 * c)]),
                    _V(z, 0, [(3 * c, 3), (1, 3 * c)]),
                    _V(tin, 9 * c, [(0, 3), (1, 3 * c)]),
                )
                st["w"] = w

            def col3_e1(st):
                # DVE f32: e1[g] = sum_j w[3g+j]
                c, w = st["c"], st["w"]
                e1 = s1.tile([P, 3 * c], f32, tag="e1")
                V.tensor_add(_V(e1, 0, [(c, 3), (1, c)]),
                             _V(w, 0, [(3 * c, 3), (1, c)]),
                             _V(w, c, [(3 * c, 3), (1, c)]))
                V.tensor_add(_V(e1, 0, [(c, 3), (1, c)]),
                             _V(e1, 0, [(c, 3), (1, c)]),
                             _V(w, 2 * c, [(3 * c, 3), (1, c)]))
                st["e1"] = e1

            def det_recip(st):
                # DVE: tm = a16 . z16[3..6] (fp16 2x); recip f32
                c, a16, z16 = st["c"], st["a16"], st["z16"]
                tm = s1.tile([P, 3 * c], f16, tag="tm")
                V.tensor_mul(_V(tm, 0, [(1, 3 * c)]),
                             _V(a16, 0, [(1, 3 * c)]),
                             _V(z16, 3 * c, [(1, 3 * c)]))
                d1 = s1.tile([P, c], f16, tag="d1")
                det = s1.tile([P, c], f32, tag="det")
                rdet = s1.tile([P, c], f32, tag="rdet")
                V.tensor_add(d1[:], _V(tm, 0, [(1, c)]), _V(tm, c, [(1, c)]))
                V.tensor_add(det[:], d1[:], _V(tm, 2 * c, [(1, c)]))
                V.reciprocal_approx_fast(rdet[:], det[:])
                st["rdet"] = rdet

            def act_rd3(st):
                c, rdet = st["c"], st["rdet"]
                rd3 = s1.tile([P, 3 * c], f16, tag="rd3")
                S.copy(_V(rd3, 0, [(1, c)]), _V(rdet, 0, [(1, c)]))
                S.copy(_V(rd3, c, [(1, c)]), _V(rd3, 0, [(1, c)]))
                S.copy(_V(rd3, 2 * c, [(1, c)]), _V(rd3, 0, [(1, c)]))
                st["rd3"] = rd3

            def act_e1cvt(st):
                c, e1 = st["c"], st["e1"]
                e16 = s1.tile([P, 3 * c], f16, tag="e16")
                S.copy(_V(e16, 0, [(1, 3 * c)]), _V(e1, 0, [(1, 3 * c)]))
                st["e16"] = e16

            def inv_scale(st):
                # DVE fp16 2x: inv16 = z16 * rd3
                c, z16, rd3 = st["c"], st["z16"], st["rd3"]
                inv16 = iv.tile([P, 9 * c], f16, tag="inv16")
                V.tensor_mul(_V(inv16, 0, [(3 * c, 3), (1, 3 * c)]),
                             _V(z16, 0, [(3 * c, 3), (1, 3 * c)]),
                             _V(rd3, 0, [(0, 3), (1, 3 * c)]))
                st["inv16"] = inv16

            def c3_scale(st):
                # DVE fp16 2x: c3 = e16 * rd3
                c, e16, rd3 = st["c"], st["e16"], st["rd3"]
                c3 = s1.tile([P, 3 * c], f16, tag="c3")
                V.tensor_mul(_V(c3, 0, [(1, 3 * c)]),
                             _V(e16, 0, [(1, 3 * c)]),
                             _V(rd3, 0, [(1, 3 * c)]))
                st["c3"] = c3

            def act_out(st):
                # inv rows 0..8 convert + fly immediately; c3 rows follow
                c, n, inv16, c3 = st["c"], st["n"], st["inv16"], st["c3"]
                tout = io.tile([P, 12 * c], f32, tag="tout")
                S.copy(_V(tout, 0, [(1, 9 * c)]), _V(inv16, 0, [(1, 9 * c)]))
                S.dma_start(out=dram_out_ap(bases[n], c, 0, 9),
                            in_=_V(tout, 0, [(1, 9 * c)]))
                S.copy(_V(tout, 9 * c, [(1, 3 * c)]), _V(c3, 0, [(1, 3 * c)]))
                S.dma_start(out=dram_out_ap(bases[n], c, 9, 3),
                            in_=_V(tout, 9 * c, [(1, 3 * c)]))
                st["tout"] = tout

            nch = len(chunks)
            states = [None] * nch
            states[0] = prep(0)
            wrap(states[0])
            for n in range(nch):
                st = states[n]
                if n + 1 < nch:
                    states[n + 1] = prep(n + 1)
                cross(st)
                tneg(st)
                conv_z(st)
                col3_w(st)
                det_recip(st)
                act_rd3(st)
                col3_e1(st)
                act_e1cvt(st)
                if n + 1 < nch:
                    wrap(states[n + 1])
                inv_scale(st)
                c3_scale(st)
                act_out(st)

    return nc


_CACHE = {}


def _get_nc():
    if "nc" not in _CACHE:
        nc = build_nc()
        nc.finalize()
        _CACHE["nc"] = nc
    return _CACHE["nc"]


def _prep_inputs(trf):
    x = np.asarray(trf, dtype=np.float32).reshape(B, 3, 4).copy()
    x[:, 0, 0] += 1.0
    x[:, 1, 1] += 1.0
    x[:, 2, 2] += 1.0
    xt = x.reshape(NCORES, BL, 3, 4).transpose(0, 3, 2, 1)[:, [1, 2, 0, 3]]
    return np.ascontiguousarray(xt.reshape(NCORES, 12, BL))


def _decode_outputs(outs):
    inv = outs[:, :9].reshape(NCORES, 3, 3, BL)
    col3 = outs[:, 9:12]
    res = np.empty((NCORES, BL, 3, 4), np.float32)
    res[..., :3] = inv.transpose(0, 3, 1, 2)
    res[..., 3] = col3.transpose(0, 2, 1)
    return res.reshape(B, 3, 4)


def run(trf, trace=False, **spmd_kwargs):
    from concourse.bass_utils import run_bass_kernel_spmd

    xin = _prep_inputs(trf)
    in_maps = [{"trf": xin[i]} for i in range(NCORES)]
    nc = _get_nc()
    res = run_bass_kernel_spmd(
        nc, in_maps, list(range(NCORES)), trace=trace, **spmd_kwargs
    )
    outs = np.stack([np.asarray(res.results[i]["out"]) for i in range(NCORES)])
    return _decode_outputs(outs), res


def kernel(trf):
    return run(trf)[0]


# revision 36
# speedup vs baseline: 1.0009x; 1.0009x over previous
"""Trainium2 Bass kernel (final): batched inverse of homogeneous affines.

DVE-saturated design (HW-traced: DVE ~95% duty, every op at its cost-model
speed, fp16 ops confirmed in 2x perf mode).
  - cross products / z sub / col3 dot / e1 stay f32: fp16 input rounding
    breaks the pointwise rel-err gate on cancellation-heavy cofactors
    (measured rel.max 4.4 with fp16 z on the fixed seed)
  - fp16 2x mode where the error stays relative per element: det dot,
    inv = z*rdet, c3 = e1*rdet (total rel.max 6.3e-3, 3x under the gate)
  - ACT does wrap-row copy, tneg, f32<->fp16 conversions, rdet
    replication, and issues output DMA on its own HWDGE ring
  - GpSimd unused: it shares the DVE SBUF port, so elementwise offload
    there is fully serialized with DVE (measured) - strictly negative
  - stride-0 broadcast APs fuse the w and inv scale muls (2x retained)
DVE FIFO order is a dependency ladder (cross, sub, w, tm, d1, det, recip,
e1, inv16, c3) so ACT work never stalls it; ~20% run-to-run clock
throttling exists on the part - compare runs via per-op durations.
"""

import numpy as np

B = 4_194_304
NCORES = 8
BL = B // NCORES
P = 128
CHUNKS = [48, 160, 320] + [448] * 7 + [240, 192]  # sums to BL/P = 4096
assert sum(CHUNKS) == BL // P


def _V(base_ap, off, dims):
    import concourse.bass as bass

    return bass.AP(
        base_ap.tensor,
        base_ap.offset + off,
        [list(base_ap.ap[0])] + [[int(s), int(n)] for s, n in dims],
    )


def build_nc(bl=BL, chunks=None):
    import concourse.bass as bass
    import concourse.bacc as bacc
    import concourse.mybir as mybir
    from concourse.tile import TileContext

    chunks = chunks or CHUNKS
    assert sum(chunks) * P == bl
    f32 = mybir.dt.float32
    f16 = mybir.dt.float16

    nc = bacc.Bacc()
    trf = nc.declare_dram_parameter("trf", [12, bl], f32, isOutput=False)
    out = nc.declare_dram_parameter("out", [12, bl], f32, isOutput=True)
    V = nc.vector
    S = nc.scalar

    bases = []
    acc = 0
    for c in chunks:
        bases.append(acc)
        acc += P * c

    def dram_in_ap(base, c, row0, nrows):
        return bass.AP(trf.ap().tensor, base + row0 * bl,
                       [[c, P], [bl, nrows], [1, c]])

    def dram_out_ap(base, c, row0, nrows):
        return bass.AP(out.ap().tensor, base + row0 * bl,
                       [[c, P], [bl, nrows], [1, c]])

    with TileContext(nc) as tc:
        with (
            tc.tile_pool(name="io", bufs=2) as io,
            tc.tile_pool(name="zz", bufs=2) as zz,
            tc.tile_pool(name="s1", bufs=1) as s1,
            tc.tile_pool(name="iv", bufs=1) as iv,
        ):

            def prep(n):
                c = chunks[n]
                tin = io.tile([P, 15 * c], f32, tag="tin")
                nc.sync.dma_start(out=_V(tin, 0, [(1, 9 * c)]),
                                  in_=dram_in_ap(bases[n], c, 0, 9))
                # rows 9..11 = DRAM rows 0..2 again (colpos_3 wrap) so the
                # cross g=2 operand needs no ACT round trip
                nc.sync.dma_start(out=_V(tin, 9 * c, [(1, 3 * c)]),
                                  in_=dram_in_ap(bases[n], c, 0, 3))
                nc.sync.dma_start(out=_V(tin, 12 * c, [(1, 3 * c)]),
                                  in_=dram_in_ap(bases[n], c, 9, 3))
                return {"tin": tin, "c": c, "n": n}

            def wrap(st):
                # ACT: a16 = fp16(col1) for the 2x det dot
                c, tin = st["c"], st["tin"]
                a16 = s1.tile([P, 3 * c], f16, tag="a16")
                S.copy(_V(a16, 0, [(1, 3 * c)]), _V(tin, 0, [(1, 3 * c)]))
                st["a16"] = a16

            def cross(st):
                # DVE f32: z[3g+j] = (colpos_g x colpos_{g+1})[j]
                # positive products land in the z tile; sub is in-place
                c, tin = st["c"], st["tin"]
                ng = s1.tile([P, 9 * c], f32, tag="ngw")
                z = zz.tile([P, 9 * c], f32, tag="z")
                for j in range(3):
                    V.tensor_mul(
                        _V(z, j * c, [(3 * c, 3), (1, c)]),
                        _V(tin, ((j + 1) % 3) * c, [(3 * c, 3), (1, c)]),
                        _V(tin, (3 + (j + 2) % 3) * c, [(3 * c, 3), (1, c)]),
                    )
                for j in range(3):
                    V.tensor_mul(
                        _V(ng, j * c, [(3 * c, 3), (1, c)]),
                        _V(tin, ((j + 2) % 3) * c, [(3 * c, 3), (1, c)]),
                        _V(tin, (3 + (j + 1) % 3) * c, [(3 * c, 3), (1, c)]),
                    )
                V.tensor_sub(_V(z, 0, [(1, 9 * c)]),
                             _V(z, 0, [(1, 9 * c)]),
                             _V(ng, 0, [(1, 9 * c)]))
                st["z"] = z

            def tneg(st):
                # ACT: rows 9..11 (wrap copy, dead after cross) := -t  (f32)
                c, tin = st["c"], st["tin"]
                S.mul(_V(tin, 9 * c, [(1, 3 * c)]),
                      _V(tin, 12 * c, [(1, 3 * c)]), -1.0)

            def conv_z(st):
                c, z = st["c"], st["z"]
                z16 = s1.tile([P, 9 * c], f16, tag="z16")
                S.copy(_V(z16, 0, [(1, 9 * c)]), _V(z, 0, [(1, 9 * c)]))
                st["z16"] = z16

            def col3_w(st):
                # DVE f32: w = z * (-t)
                c, tin, z = st["c"], st["tin"], st["z"]
                w = s1.tile([P, 9 * c], f32, tag="ngw")
                V.tensor_mul(
                    _V(w, 0, [(3 * c, 3), (1, 3 * c)]),
                    _V(z, 0, [(3 * c, 3), (1, 3 * c)]),
                    _V(tin, 9 * c, [(0, 3), (1, 3 * c)]),
                )
                st["w"] = w

            def col3_e1(st):
                # DVE f32: e1[g] = sum_j w[3g+j]
                c, w = st["c"], st["w"]
                e1 = s1.tile([P, 3 * c], f32, tag="e1")
                V.tensor_add(_V(e1, 0, [(c, 3), (1, c)]),
                             _V(w, 0, [(3 * c, 3), (1, c)]),
                             _V(w, c, [(3 * c, 3), (1, c)]))
                V.tensor_add(_V(e1, 0, [(c, 3), (1, c)]),
                             _V(e1, 0, [(c, 3), (1, c)]),
                             _V(w, 2 * c, [(3 * c, 3), (1, c)]))
                st["e1"] = e1

            def det_recip(st):
                # DVE: tm = a16 . z16[3..6] (fp16 2x); recip f32
                c, a16, z16 = st["c"], st["a16"], st["z16"]
                tm = s1.tile([P, 3 * c], f16, tag="tm")
                V.tensor_mul(_V(tm, 0, [(1, 3 * c)]),
                             _V(a16, 0, [(1, 3 * c)]),
                             _V(z16, 3 * c, [(1, 3 * c)]))
                d1 = s1.tile([P, c], f16, tag="d1")
                det = s1.tile([P, c], f32, tag="det")
                rdet = s1.tile([P, c], f32, tag="rdet")
                V.tensor_add(d1[:], _V(tm, 0, [(1, c)]), _V(tm, c, [(1, c)]))
                V.tensor_add(det[:], d1[:], _V(tm, 2 * c, [(1, c)]))
                V.reciprocal_approx_fast(rdet[:], det[:])
                st["rdet"] = rdet

            def act_rd3(st):
                c, rdet = st["c"], st["rdet"]
                rd3 = s1.tile([P, 3 * c], f16, tag="rd3")
                S.copy(_V(rd3, 0, [(1, c)]), _V(rdet, 0, [(1, c)]))
                S.copy(_V(rd3, c, [(1, c)]), _V(rd3, 0, [(1, c)]))
                S.copy(_V(rd3, 2 * c, [(1, c)]), _V(rd3, 0, [(1, c)]))
                st["rd3"] = rd3

            def act_e1cvt(st):
                c, e1 = st["c"], st["e1"]
                e16 = s1.tile([P, 3 * c], f16, tag="e16")
                S.copy(_V(e16, 0, [(1, 3 * c)]), _V(e1, 0, [(1, 3 * c)]))
                st["e16"] = e16

            def inv_scale(st):
                # DVE fp16 2x: inv16 = z16 * rd3
                c, z16, rd3 = st["c"], st["z16"], st["rd3"]
                inv16 = iv.tile([P, 9 * c], f16, tag="inv16")
                V.tensor_mul(_V(inv16, 0, [(3 * c, 3), (1, 3 * c)]),
                             _V(z16, 0, [(3 * c, 3), (1, 3 * c)]),
                             _V(rd3, 0, [(0, 3), (1, 3 * c)]))
                st["inv16"] = inv16

            def c3_scale(st):
                # DVE fp16 2x: c3 = e16 * rd3
                c, e16, rd3 = st["c"], st["e16"], st["rd3"]
                c3 = s1.tile([P, 3 * c], f16, tag="c3")
                V.tensor_mul(_V(c3, 0, [(1, 3 * c)]),
                             _V(e16, 0, [(1, 3 * c)]),
                             _V(rd3, 0, [(1, 3 * c)]))
                st["c3"] = c3

            def act_out(st):
                # inv rows 0..8 convert + fly immediately; c3 rows follow
                c, n, inv16, c3 = st["c"], st["n"], st["inv16"], st["c3"]
                tout = io.tile([P, 12 * c], f32, tag="tout")
                S.copy(_V(tout, 0, [(1, 9 * c)]), _V(inv16, 0, [(1, 9 * c)]))
                S.dma_start(out=dram_out_ap(bases[n], c, 0, 9),
                            in_=_V(tout, 0, [(1, 9 * c)]))
                S.copy(_V(tout, 9 * c, [(1, 3 * c)]), _V(c3, 0, [(1, 3 * c)]))
                nc.sync.dma_start(out=dram_out_ap(bases[n], c, 9, 3),
                                  in_=_V(tout, 9 * c, [(1, 3 * c)]))
                st["tout"] = tout

            nch = len(chunks)
            states = [None] * nch
            states[0] = prep(0)
            wrap(states[0])
            for n in range(nch):
                st = states[n]
                if n + 1 < nch:
                    states[n + 1] = prep(n + 1)
                cross(st)
                tneg(st)
                conv_z(st)
                col3_w(st)
                det_recip(st)
                act_rd3(st)
                col3_e1(st)
                act_e1cvt(st)
                if n + 1 < nch:
                    wrap(states[n + 1])
                inv_scale(st)
                c3_scale(st)
                act_out(st)

    return nc


_CACHE = {}


def _get_nc():
    if "nc" not in _CACHE:
        nc = build_nc()
        nc.finalize()
        _CACHE["nc"] = nc
    return _CACHE["nc"]


def _prep_inputs(trf):
    x = np.asarray(trf, dtype=np.float32).reshape(B, 3, 4).copy()
    x[:, 0, 0] += 1.0
    x[:, 1, 1] += 1.0
    x[:, 2, 2] += 1.0
    xt = x.reshape(NCORES, BL, 3, 4).transpose(0, 3, 2, 1)[:, [1, 2, 0, 3]]
    return np.ascontiguousarray(xt.reshape(NCORES, 12, BL))


def _decode_outputs(outs):
    inv = outs[:, :9].reshape(NCORES, 3, 3, BL)
    col3 = outs[:, 9:12]
    res = np.empty((NCORES, BL, 3, 4), np.float32)
    res[..., :3] = inv.transpose(0, 3, 1, 2)
    res[..., 3] = col3.transpose(0, 2, 1)
    return res.reshape(B, 3, 4)


def run(trf, trace=False, **spmd_kwargs):
    from concourse.bass_utils import run_bass_kernel_spmd

    xin = _prep_inputs(trf)
    in_maps = [{"trf": xin[i]} for i in range(NCORES)]
    nc = _get_nc()
    res = run_bass_kernel_spmd(
        nc, in_maps, list(range(NCORES)), trace=trace, **spmd_kwargs
    )
    outs = np.stack([np.asarray(res.results[i]["out"]) for i in range(NCORES)])
    return _decode_outputs(outs), res


def kernel(trf):
    return run(trf)[0]
